# revision 1
# baseline (speedup 1.0000x reference)
"""Trainium2 Bass kernel for BondingGraphGNN (gnn_message_passing).

Model (see reference):
  h = relu(x @ W_emb)
  4x: m = h @ W_msg[i]; agg = scatter_add(m[src] -> dst); h = GRU(agg, h)
  h = relu(h); pooled = segment_mean(h, batch); out = softplus(relu(pooled@W1+b1)@W2+b2)

Distribution: graph-aligned node sharding across 8 cores. Per step each core
computes messages for its nodes (node-major bf16 table shard), AllGathers the
full table to DRAM, dma_gathers the rows for its incoming edges (grouped by
128-dst-node groups, lo/hi split for int16 indices), segment-sums them on the
tensor engine via on-chip-built one-hot S matrices, and runs the GRU locally.
Readout: transpose + one-hot graph matmul pooling + tiny MLP per core.

Host side does only data layout (shard/sort/pad/index building) — all float
math runs on device.
"""

import os
import numpy as np

# the trimmed axon package in some containers lacks the NTFF profile hook
# module; stub it so run_bass_kernel_spmd(trace=True) degrades gracefully.
import sys as _sys, types as _types
try:
    import antenv.axon_hooks  # noqa: F401
except Exception:
    _m = _types.ModuleType("antenv.axon_hooks")
    _m.get_axon_ntff_profile_hook = lambda: None
    _sys.modules["antenv.axon_hooks"] = _m

import concourse.bacc as bacc
import concourse.bass as bass
import concourse.mybir as mybir
import concourse.tile as tile
from concourse.bass_utils import run_bass_kernel_spmd

F32 = mybir.dt.float32
BF16 = mybir.dt.bfloat16
I16 = mybir.dt.int16
AF = mybir.ActivationFunctionType

N_NODES = 50000
N_EDGES = 800000
FEAT = 90
H = 128
STEPS = 4
N_GRAPHS = 100
N_CORES = 8
G_PAD = 16          # max graphs per core (padded)
GROUPS_PER_CHUNK = 4
PAD_SLOT = 255.0    # sentinel slot -> all-zero S row

LAST_RESULTS = {}   # stash for test.py (exec time etc)


# ----------------------------------------------------------------------------
# host-side layout
# ----------------------------------------------------------------------------

def _preprocess(x, edge_index, batch):
    batch = np.asarray(batch, np.int64)
    src = np.asarray(edge_index[0], np.int64)
    dst = np.asarray(edge_index[1], np.int64)

    counts = np.bincount(batch, minlength=N_GRAPHS)
    cum = np.concatenate([[0], np.cumsum(counts)])  # [101]
    # graph-aligned core boundaries (nodes are sorted by batch)
    targets = [round(N_NODES * c / N_CORES) for c in range(N_CORES + 1)]
    gsplit = [0]
    for c in range(1, N_CORES):
        g = int(np.argmin(np.abs(cum - targets[c])))
        g = max(g, gsplit[-1])
        gsplit.append(g)
    gsplit.append(N_GRAPHS)
    bounds = np.array([cum[g] for g in gsplit], np.int64)  # node bounds per core
    n_c = np.diff(bounds)
    assert (n_c > 0).all()
    n_graphs_c = np.diff(np.array(gsplit))
    assert n_graphs_c.max() <= G_PAD, f"graphs per core {n_graphs_c.max()} > {G_PAD}"

    n_pad = int(np.ceil(n_c.max() / 512) * 512)
    table_rows = N_CORES * n_pad
    table_split = table_rows // 2
    assert table_split <= 32767 and table_rows - table_split <= 32767

    core_of = np.searchsorted(bounds, np.arange(N_NODES), side="right") - 1

    d_core = core_of[dst]
    d_local = dst - bounds[d_core]
    grp = d_local // 128
    slot = d_local % 128
    tpos = core_of[src] * n_pad + (src - bounds[core_of[src]])
    is_lo = tpos < table_split
    idxval = np.where(is_lo, tpos, tpos - table_split).astype(np.int64)

    n_groups = n_pad // 128
    # per (core, group, half) edge counts -> tile budgets (uniform across cores)
    cnt = np.zeros((N_CORES, n_groups, 2), np.int64)
    np.add.at(cnt, (d_core, grp, 1 - is_lo.astype(np.int64)), 1)
    budget = np.ceil(cnt.max(axis=0) / 128).astype(np.int64)  # [n_groups, 2]

    # chunk structure: chunks of GROUPS_PER_CHUNK groups;
    # tile order: [chunk0: g0lo.. g3lo, g0hi.. g3hi][chunk1: ...]
    chunks = []
    t = 0
    for c0 in range(0, n_groups, GROUPS_PER_CHUNK):
        gs = list(range(c0, min(c0 + GROUPS_PER_CHUNK, n_groups)))
        lo0 = t
        lo_off = {}
        for g in gs:
            lo_off[g] = t
            t += int(budget[g, 0])
        hi0 = t
        hi_off = {}
        for g in gs:
            hi_off[g] = t
            t += int(budget[g, 1])
        chunks.append(dict(groups=gs, t_lo=lo0, n_lo=hi0 - lo0,
                           t_hi=hi0, n_hi=t - hi0,
                           lo_off=lo_off, hi_off=hi_off))
    t_tot = t

    # per-core edge placement
    order = np.lexsort((1 - is_lo, grp, d_core))  # by core, group, lo-first
    idx_arr = np.zeros((N_CORES, 16, t_tot * 8), np.int16)
    slot_arr = np.full((N_CORES, 128, t_tot), PAD_SLOT, np.float32)

    s_core = d_core[order]
    s_grp = grp[order]
    s_lo = is_lo[order]
    s_idx = idxval[order]
    s_slot = slot[order]
    # position of each edge within its (core, group, half) run
    # compute run starts via cumulative counts
    e_ptr = 0
    for c in range(N_CORES):
        c_end = e_ptr + int((d_core == c).sum())
        seg = slice(e_ptr, c_end)
        e_ptr2 = seg.start
        for ch in chunks:
            for half in (0, 1):
                for g in ch["groups"]:
                    n_e = int(cnt[c, g, half])
                    if n_e == 0:
                        continue
                    t0g = (ch["lo_off"] if half == 0 else ch["hi_off"])[g]
                    tg0_gather = ch["t_lo"] if half == 0 else ch["t_hi"]
                    ed = slice(e_ptr2, e_ptr2 + n_e)
                    e_ptr2 += n_e
                    j = np.arange(n_e)
                    tt = t0g + j // 128          # global tile
                    pp = j % 128                 # row in tile
                    pos = (tt - tg0_gather) * 128 + pp   # position within gather
                    idx_arr[c, pos % 16, tg0_gather * 8 + pos // 16] = s_idx[ed]
                    slot_arr[c, pp, tt] = s_slot[ed]
        e_ptr = c_end

    # per-core inputs
    per_core = []
    for c in range(N_CORES):
        nc_nodes = int(n_c[c])
        xT = np.zeros((FEAT, n_pad), np.float32)
        xT[:, :nc_nodes] = np.asarray(x[bounds[c]:bounds[c + 1]], np.float32).T
        gloc = (batch[bounds[c]:bounds[c + 1]] - gsplit[c]).astype(np.int64)
        nt = n_pad // 128
        gmat = np.zeros((128, nt * G_PAD), np.float32)
        node_ids = np.arange(nc_nodes)
        gmat[node_ids % 128, (node_ids // 128) * G_PAD + gloc] = 1.0
        cc = counts[gsplit[c]:gsplit[c + 1]].astype(np.float32)
        invc = np.zeros((G_PAD, 1), np.float32)
        invc[:len(cc), 0] = 1.0 / np.maximum(cc, 1.0)
        per_core.append(dict(
            xT=xT,
            idx=np.tile(idx_arr[c], (8, 1)),
            slot=slot_arr[c],
            gmat=gmat,
            invc=invc,
        ))

    meta = dict(n_pad=n_pad, t_tot=t_tot, chunks=chunks, budget=budget,
                table_rows=table_rows, table_split=table_split,
                n_groups=n_groups, bounds=bounds, gsplit=gsplit,
                n_graphs_c=n_graphs_c)
    return per_core, meta


# ----------------------------------------------------------------------------
# device program
# ----------------------------------------------------------------------------

def _build(meta):
    DBG_STEPS = int(os.environ.get("K_STEPS", STEPS))
    DBG_NO_MSG = bool(int(os.environ.get("K_NO_MSG", "0")))
    DBG_NO_AGG = bool(int(os.environ.get("K_NO_AGG", "0")))
    DBG_NO_GRU = bool(int(os.environ.get("K_NO_GRU", "0")))
    DBG_NO_POOL = bool(int(os.environ.get("K_NO_POOL", "0")))
    DBG_AGG_LVL = int(os.environ.get("K_AGG_LVL", "4"))
    n_pad = meta["n_pad"]
    t_tot = meta["t_tot"]
    chunks = meta["chunks"]
    budget = meta["budget"]
    table_rows = meta["table_rows"]
    table_split = meta["table_split"]
    nt = n_pad // 128          # 128-node tiles
    n_ch512 = n_pad // 512     # GRU chunks

    nc = bacc.Bacc("TRN2", target_bir_lowering=False, debug=False,
                   num_devices=N_CORES)

    # inputs
    d_xT = nc.dram_tensor("xT", [FEAT, n_pad], F32, kind="ExternalInput")
    d_idx = nc.dram_tensor("idx", [128, t_tot * 8], I16, kind="ExternalInput")
    d_slot = nc.dram_tensor("slot", [128, t_tot], F32, kind="ExternalInput")
    d_gmat = nc.dram_tensor("gmat", [128, nt * G_PAD], F32, kind="ExternalInput")
    d_invc = nc.dram_tensor("invc", [G_PAD, 1], F32, kind="ExternalInput")
    d_iota = nc.dram_tensor("iota", [1, 128], F32, kind="ExternalInput")
    d_ident = nc.dram_tensor("ident", [128, 128], F32, kind="ExternalInput")
    d_wemb = nc.dram_tensor("wemb", [FEAT, H], F32, kind="ExternalInput")
    d_wmsg = nc.dram_tensor("wmsg", [STEPS, H, H], F32, kind="ExternalInput")
    d_wih = nc.dram_tensor("wih", [H, 3 * H], F32, kind="ExternalInput")
    d_whh = nc.dram_tensor("whh", [H, 3 * H], F32, kind="ExternalInput")
    d_bihT = nc.dram_tensor("bihT", [H, 3], F32, kind="ExternalInput")
    d_bhhT = nc.dram_tensor("bhhT", [H, 3], F32, kind="ExternalInput")
    d_w1 = nc.dram_tensor("w1", [H, H], F32, kind="ExternalInput")
    d_b1 = nc.dram_tensor("b1", [H, 1], F32, kind="ExternalInput")
    d_w2 = nc.dram_tensor("w2", [H, 1], F32, kind="ExternalInput")
    d_b2 = nc.dram_tensor("b2", [1, 1], F32, kind="ExternalInput")
    d_out = nc.dram_tensor("out", [1, G_PAD], F32, kind="ExternalOutput")

    with tile.TileContext(nc) as tc:
        with (
            tc.tile_pool(name="persist", bufs=1) as P,
            tc.tile_pool(name="dram", bufs=1, space="DRAM") as DR,
            tc.tile_pool(name="stg", bufs=2) as STG,
            tc.tile_pool(name="epool", bufs=2) as EP,
            tc.tile_pool(name="spool", bufs=6) as SP,
            tc.tile_pool(name="mpool", bufs=4) as MP,
            tc.tile_pool(name="gpool", bufs=2) as GP,
            tc.tile_pool(name="ps_m", bufs=2, space="PSUM") as PS_M,
            tc.tile_pool(name="ps_agg", bufs=2, space="PSUM") as PS_AGG,
            tc.tile_pool(name="ps_gru", bufs=4, space="PSUM") as PS_GRU,
        ):
            PS_TR = PS_M
            shard_bufs = [DR.tile([n_pad, H], BF16, name=f"m_shard{s_}")
                          for s_ in range(2)]
            table_bufs = [DR.tile([table_rows, H], BF16, addr_space="Shared",
                                  name=f"m_table{s_}") for s_ in range(STEPS)]

            # ---------------- constants / weights ----------------
            slot_all = P.tile([128, t_tot], F32, name="slot_all")
            nc.sync.dma_start(out=slot_all[:], in_=d_slot[:, :])

            iota_f = STG.tile([128, 128], F32, name="iota_f", tag="stgf")
            nc.sync.dma_start(out=iota_f[:], in_=d_iota.ap().to_broadcast([128, 128]))
            iota_b = P.tile([128, 128], BF16, name="iota_b")
            nc.vector.tensor_copy(iota_b[:], iota_f[:])

            ident_f = STG.tile([128, 128], F32, name="ident_f", tag="stgf")
            nc.sync.dma_start(out=ident_f[:], in_=d_ident[:, :])
            ident_b = P.tile([128, 128], BF16, name="ident_b")
            nc.vector.tensor_copy(ident_b[:], ident_f[:])

            def load_bf(dram_ap, shape, name):
                tf = STG.tile(shape, F32, name=name + "_f", tag="stgf")
                nc.sync.dma_start(out=tf[:], in_=dram_ap)
                tb = P.tile(shape, BF16, name=name + "_b")
                nc.scalar.activation(tb[:], tf[:], AF.Copy)
                return tb

            wemb_b = load_bf(d_wemb[:, :], [FEAT, H], "wemb")
            wmsg_b = [load_bf(d_wmsg[s, :, :], [H, H], f"wmsg{s}")
                      for s in range(STEPS)]
            wih_b = load_bf(d_wih[:, :], [H, 3 * H], "wih")
            whh_b = load_bf(d_whh[:, :], [H, 3 * H], "whh")
            w1_b = load_bf(d_w1[:, :], [H, H], "w1")
            w2_b = load_bf(d_w2[:, :], [H, 1], "w2")

            bih = P.tile([H, 3], F32, name="bih")
            nc.sync.dma_start(out=bih[:], in_=d_bihT[:, :])
            bhh = P.tile([H, 3], F32, name="bhh")
            nc.sync.dma_start(out=bhh[:], in_=d_bhhT[:, :])
            bsum = P.tile([H, 3], F32, name="bsum")
            nc.vector.tensor_add(bsum[:], bih[:], bhh[:])
            b1t = P.tile([H, 1], F32, name="b1t")
            nc.sync.dma_start(out=b1t[:], in_=d_b1[:, :])
            b2t = P.tile([1, 1], F32, name="b2t")
            nc.sync.dma_start(out=b2t[:], in_=d_b2[:, :])
            invc_t = P.tile([G_PAD, 1], F32, name="invc_t")
            nc.sync.dma_start(out=invc_t[:], in_=d_invc[:, :])
            gmat_f = STG.tile([128, nt * G_PAD], F32, name="gmat_f", tag="stgf")
            nc.sync.dma_start(out=gmat_f[:], in_=d_gmat[:, :])
            gmat_b = P.tile([128, nt * G_PAD], BF16, name="gmat_b")
            nc.scalar.activation(gmat_b[:], gmat_f[:], AF.Copy)

            # state
            hA = P.tile([128, n_pad], F32, name="hA")
            hB = P.tile([128, n_pad], F32, name="hB")
            h_bf_parts = []
            for j in range(n_ch512):
                hb_t = P.tile([128, 512], BF16, name=f"h_bf{j}")
                h_bf_parts.append(hb_t)
            m_all = P.tile([128, n_pad], BF16, name="m_all")
            aggT_parts = []
            for j in range(n_ch512):
                ap_t = P.tile([128, 512], BF16, name=f"aggT{j}")
                nc.vector.memset(ap_t[:], 0.0)
                aggT_parts.append(ap_t)

            # ---------------- embedding ----------------
            for j in range(n_ch512):
                sl = slice(j * 512, (j + 1) * 512)
                xT_f = STG.tile([FEAT, 512], F32, name="xT_f", tag="stgx")
                nc.sync.dma_start(out=xT_f[:], in_=d_xT[:, sl])
                xT_b = STG.tile([FEAT, 512], BF16, name="xT_b", tag="stgxb")
                nc.scalar.activation(xT_b[:], xT_f[:], AF.Copy)
                pe = PS_GRU.tile([128, 512], F32, name="pe_emb", tag="pgru")
                nc.tensor.matmul(pe[:], lhsT=wemb_b[:, :], rhs=xT_b[:, :],
                                 start=True, stop=True)
                nc.scalar.activation(hA[:, sl], pe[:], AF.Relu)

            # ---------------- message-passing steps ----------------
            for step in range(DBG_STEPS):
                h_cur = hA if step % 2 == 0 else hB
                h_nxt = hB if step % 2 == 0 else hA
                shard = shard_bufs[step % 2]
                table = table_bufs[step]

                for j in range(n_ch512):
                    nc.scalar.activation(h_bf_parts[j][:],
                                         h_cur[:, j * 512:(j + 1) * 512], AF.Copy)

                if DBG_NO_MSG:
                    nc.vector.tensor_copy(h_nxt[:], h_cur[:])
                    continue
                # messages: m[node, feat] bf16 -> dram shard
                for t in range(nt):
                    hb = h_bf_parts[t // 4][:, (t % 4) * 128:(t % 4 + 1) * 128]
                    pm = PS_M.tile([128, 128], F32, name="pm", tag="pmisc")
                    nc.tensor.matmul(pm[:], lhsT=hb,
                                     rhs=wmsg_b[step][:, :], start=True, stop=True)
                    nc.scalar.activation(m_all[:, t * 128:(t + 1) * 128],
                                         pm[:], AF.Copy)
                # node n = 128*t + p lives at m_all[p, t*128 : t*128+128]
                nc.sync.dma_start(
                    out=shard.rearrange("(a p) b -> p a b", p=128),
                    in_=m_all[:].rearrange("p (a b) -> p a b", b=128))

                nc.gpsimd.collective_compute(
                    "AllGather", mybir.AluOpType.bypass,
                    ins=[shard.opt()], outs=[table.opt()],
                    replica_groups=[list(range(N_CORES))],
                )
                tab_lo = table[0:table_split, :]
                tab_hi = table[table_split:table_rows, :]

                if DBG_NO_AGG:
                    nc.vector.tensor_copy(h_nxt[:], h_cur[:])
                    continue
                # aggregation
                for ci, ch in enumerate(chunks):
                    n_tiles_ch = ch["n_lo"] + ch["n_hi"]
                    if n_tiles_ch == 0:
                        continue
                    E_lo = E_hi = None
                    if ch["n_lo"] > 0:
                        nidx = ch["n_lo"] * 128
                        ix_lo = EP.tile([128, ch["n_lo"] * 8], I16,
                                        name="ix_lo", tag="ix_lo")
                        nc.sync.dma_start(
                            out=ix_lo[:],
                            in_=d_idx[:, ch["t_lo"] * 8:(ch["t_lo"] + ch["n_lo"]) * 8])
                        E_lo = EP.tile([128, ch["n_lo"], 128], BF16,
                                       name="E_lo", tag="E_lo")
                        nc.gpsimd.dma_gather(E_lo[:], tab_lo, ix_lo[:],
                                             nidx, nidx, H,
                                             single_packet=False)
                    if ch["n_hi"] > 0:
                        nidx = ch["n_hi"] * 128
                        ix_hi = EP.tile([128, ch["n_hi"] * 8], I16,
                                        name="ix_hi", tag="ix_hi")
                        nc.sync.dma_start(
                            out=ix_hi[:],
                            in_=d_idx[:, ch["t_hi"] * 8:(ch["t_hi"] + ch["n_hi"]) * 8])
                        E_hi = EP.tile([128, ch["n_hi"], 128], BF16,
                                       name="E_hi", tag="E_hi")
                        nc.gpsimd.dma_gather(E_hi[:], tab_hi, ix_hi[:],
                                             nidx, nidx, H,
                                             single_packet=False)
                    if DBG_AGG_LVL < 2:
                        continue
                    for g in ch["groups"]:
                        tl = []  # (tile id, E tile, position)
                        for k in range(int(budget[g, 0])):
                            tg = ch["lo_off"][g] + k
                            tl.append((tg, E_lo, tg - ch["t_lo"]))
                        for k in range(int(budget[g, 1])):
                            tg = ch["hi_off"][g] + k
                            tl.append((tg, E_hi, tg - ch["t_hi"]))
                        if not tl:
                            continue
                        pa = PS_AGG.tile([128, 128], F32, name="pa", tag="pa")
                        for k, (tg, Et, ep) in enumerate(tl):
                            St = SP.tile([128, 128], BF16, name="St", tag="St")
                            nc.vector.tensor_scalar(
                                St[:], iota_b[:], slot_all[:, tg:tg + 1], None,
                                mybir.AluOpType.is_equal)
                            if DBG_AGG_LVL >= 3:
                                nc.tensor.matmul(pa[:], lhsT=St[:], rhs=Et[:, ep, :],
                                                 start=(k == 0), stop=(k == len(tl) - 1))
                        if DBG_AGG_LVL < 3:
                            continue
                        anm = MP.tile([128, 128], BF16, name="anm", tag="mp")
                        nc.scalar.activation(anm[:], pa[:], AF.Copy)
                        if DBG_AGG_LVL < 4:
                            g0 = g - ci * GROUPS_PER_CHUNK
                            nc.scalar.activation(
                                aggT_parts[ci][:, g0 * 128:(g0 + 1) * 128],
                                anm[:], AF.Copy)
                            continue
                        ptr = PS_TR.tile([128, 128], BF16, name="ptr", tag="pmisc")
                        nc.tensor.transpose(ptr[:], anm[:], ident_b[:])
                        g0 = g - ci * GROUPS_PER_CHUNK
                        nc.scalar.activation(
                            aggT_parts[ci][:, g0 * 128:(g0 + 1) * 128],
                            ptr[:], AF.Copy)

                # GRU
                if DBG_NO_GRU:
                    nc.vector.tensor_copy(h_nxt[:], h_cur[:])
                    continue
                for j in range(n_ch512):
                    sl = slice(j * 512, (j + 1) * 512)
                    aT = aggT_parts[j]
                    p_r = PS_GRU.tile([128, 512], F32, name="p_r", tag="pgru")
                    nc.tensor.matmul(p_r[:], lhsT=wih_b[:, 0:128], rhs=aT[:, :],
                                     start=True, stop=False)
                    nc.tensor.matmul(p_r[:], lhsT=whh_b[:, 0:128],
                                     rhs=h_bf_parts[j][:, :],
                                     start=False, stop=True)
                    p_z = PS_GRU.tile([128, 512], F32, name="p_z", tag="pgru")
                    nc.tensor.matmul(p_z[:], lhsT=wih_b[:, 128:256], rhs=aT[:, :],
                                     start=True, stop=False)
                    nc.tensor.matmul(p_z[:], lhsT=whh_b[:, 128:256],
                                     rhs=h_bf_parts[j][:, :],
                                     start=False, stop=True)
                    p_xn = PS_GRU.tile([128, 512], F32, name="p_xn", tag="pgru")
                    nc.tensor.matmul(p_xn[:], lhsT=wih_b[:, 256:384], rhs=aT[:, :],
                                     start=True, stop=True)
                    p_hn = PS_GRU.tile([128, 512], F32, name="p_hn", tag="pgru")
                    nc.tensor.matmul(p_hn[:], lhsT=whh_b[:, 256:384],
                                     rhs=h_bf_parts[j][:, :],
                                     start=True, stop=True)
                    r_t = GP.tile([128, 512], BF16, name="r_t")
                    nc.scalar.activation(r_t[:], p_r[:], AF.Sigmoid, bias=bsum[:, 0:1])
                    z_t = GP.tile([128, 512], BF16, name="z_t")
                    nc.scalar.activation(z_t[:], p_z[:], AF.Sigmoid, bias=bsum[:, 1:2])
                    hn_t = GP.tile([128, 512], BF16, name="hn_t")
                    nc.scalar.activation(hn_t[:], p_hn[:], AF.Identity,
                                         bias=bhh[:, 2:3])
                    t1 = GP.tile([128, 512], BF16, name="t1")
                    nc.vector.tensor_mul(t1[:], r_t[:], hn_t[:])
                    u_t = GP.tile([128, 512], F32, name="u_t")
                    nc.vector.tensor_add(u_t[:], t1[:], p_xn[:])
                    n_t = GP.tile([128, 512], F32, name="n_t")
                    nc.scalar.activation(n_t[:], u_t[:], AF.Tanh, bias=bih[:, 2:3])
                    d_t = GP.tile([128, 512], F32, name="d_t")
                    nc.vector.tensor_sub(d_t[:], h_cur[:, sl], n_t[:])
                    e_t = GP.tile([128, 512], F32, name="e_t")
                    nc.vector.tensor_mul(e_t[:], z_t[:], d_t[:])
                    nc.vector.tensor_add(h_nxt[:, sl], n_t[:], e_t[:])

            # ---------------- readout ----------------
            if DBG_NO_POOL:
                zz = P.tile([1, G_PAD], F32, name="zz")
                nc.vector.memset(zz[:], 0.0)
                nc.sync.dma_start(out=d_out[:, :], in_=zz[:])
            else:
                h_fin = hA if DBG_STEPS % 2 == 0 else hB
                for j in range(n_ch512):
                    nc.scalar.activation(h_bf_parts[j][:],
                                         h_fin[:, j * 512:(j + 1) * 512], AF.Relu)
                pp = PS_GRU.tile([G_PAD, 128], F32, name="pp", tag="pgru")
                for t in range(nt):
                    hb = h_bf_parts[t // 4][:, (t % 4) * 128:(t % 4 + 1) * 128]
                    ptr2 = PS_TR.tile([128, 128], BF16, name="ptr2", tag="pmisc")
                    nc.tensor.transpose(ptr2[:], hb, ident_b[:])
                    hnm = MP.tile([128, 128], BF16, name="hnm", tag="mp")
                    nc.scalar.activation(hnm[:], ptr2[:], AF.Copy)
                    nc.tensor.matmul(pp[:], lhsT=gmat_b[:, t * G_PAD:(t + 1) * G_PAD],
                                     rhs=hnm[:], start=(t == 0), stop=(t == nt - 1))
                pooled = P.tile([G_PAD, 128], BF16, name="pooled")
                nc.vector.tensor_scalar(pooled[:], pp[:], invc_t[:], None,
                                        mybir.AluOpType.mult)
                ppt = PS_TR.tile([128, G_PAD], BF16, name="ppt", tag="pmisc")
                nc.tensor.transpose(ppt[:], pooled[:], ident_b[0:G_PAD, 0:G_PAD])
                pooledT = P.tile([128, G_PAD], BF16, name="pooledT")
                nc.scalar.activation(pooledT[:], ppt[:], AF.Copy)
                pz1 = PS_M.tile([128, G_PAD], F32, name="pz1", tag="pmisc")
                nc.tensor.matmul(pz1[:], lhsT=w1_b[:, :], rhs=pooledT[:],
                                 start=True, stop=True)
                z1 = P.tile([128, G_PAD], BF16, name="z1")
                nc.scalar.activation(z1[:], pz1[:], AF.Relu, bias=b1t[:, 0:1])
                po = PS_M.tile([1, G_PAD], F32, name="po", tag="pmisc")
                nc.tensor.matmul(po[:], lhsT=w2_b[:, :], rhs=z1[:],
                                 start=True, stop=True)
                esb = P.tile([1, G_PAD], F32, name="esb")
                nc.scalar.activation(esb[:], po[:], AF.Exp, bias=b2t[:, 0:1])
                osb = P.tile([1, G_PAD], F32, name="osb")
                nc.scalar.activation(osb[:], esb[:], AF.Ln, bias=1.0)
                nc.sync.dma_start(out=d_out[:, :], in_=osb[:])

    nc.compile()
    return nc


# ----------------------------------------------------------------------------
# entry point
# ----------------------------------------------------------------------------

def kernel(x, edge_index, batch, W_emb, W_msg, W_ih, W_hh, b_ih, b_hh,
           W1, b1, W2, b2):
    x = np.asarray(x, np.float32)
    per_core, meta = _preprocess(x, edge_index, batch)
    nc = _build(meta)

    shared = dict(
        iota=np.arange(128, dtype=np.float32).reshape(1, 128),
        ident=np.eye(128, dtype=np.float32),
        wemb=np.asarray(W_emb, np.float32),
        wmsg=np.asarray(W_msg, np.float32),
        wih=np.asarray(W_ih, np.float32),
        whh=np.asarray(W_hh, np.float32),
        bihT=np.ascontiguousarray(np.asarray(b_ih, np.float32).reshape(3, H).T),
        bhhT=np.ascontiguousarray(np.asarray(b_hh, np.float32).reshape(3, H).T),
        w1=np.asarray(W1, np.float32),
        b1=np.asarray(b1, np.float32).reshape(H, 1),
        w2=np.asarray(W2, np.float32),
        b2=np.asarray(b2, np.float32).reshape(1, 1),
    )
    in_maps = []
    for c in range(N_CORES):
        m = dict(shared)
        m["xT"] = per_core[c]["xT"]
        m["idx"] = per_core[c]["idx"]
        m["slot"] = per_core[c]["slot"]
        m["gmat"] = per_core[c]["gmat"]
        m["invc"] = per_core[c]["invc"]
        in_maps.append(m)

    trace = bool(int(os.environ.get("KERNEL_TRACE", "0")))
    res = run_bass_kernel_spmd(nc, in_maps, list(range(N_CORES)), trace=trace)
    LAST_RESULTS["exec_time_ns"] = res.exec_time_ns
    LAST_RESULTS["profile_json"] = res.profile_json
    LAST_RESULTS["nc"] = nc
    LAST_RESULTS["in_maps"] = in_maps

    out = np.zeros((N_GRAPHS,), np.float32)
    gsplit = meta["gsplit"]
    for c in range(N_CORES):
        ng = gsplit[c + 1] - gsplit[c]
        out[gsplit[c]:gsplit[c + 1]] = res.results[c]["out"][0, :ng]
    return out



# revision 4
# speedup vs baseline: 26.0613x; 26.0613x over previous
"""Trainium2 Bass kernel for BondingGraphGNN (gnn_message_passing), v2.

Model (see reference):
  h = relu(x @ W_emb)
  4x: m = h @ W_msg[i]; agg = scatter_add(m[src] -> dst); h = GRU(agg, h)
  h = relu(h); pooled = segment_mean(h, batch); out = softplus(relu(pooled@W1+b1)@W2+b2)

Distribution: even node sharding (6250 nodes/core, padded). Per step each core
computes messages for its nodes (node-major bf16 table shard), AllGathers the
table to DRAM in N_AG pieces, dma_gathers rows for its incoming edges (grouped
by 128-dst-node groups), segment-sums them on the tensor engine via on-chip
one-hot S matrices (operand-swapped so the result lands feature-major), and
runs the GRU locally.

Pipelining: the table is split into N_SEG = max(N_AG, 2) source segments; the
aggregation runs one pass per segment (pass 0 overwrites aggT, later passes
add), so pass s only depends on AllGather s. In the final pass the GRU and the
NEXT step's message matmuls run chunk-by-chunk behind the aggregation, and
each next-step AllGather is issued as soon as the message tiles it needs are
done - the collective engine streams continuously instead of serializing with
compute.

Readout: per-core partial pooling + AllReduce + tiny MLP replicated on every
core (host takes core 0).
"""

import os
import numpy as np

# the trimmed axon package in some containers lacks the NTFF profile hook
# module; stub it so run_bass_kernel_spmd(trace=True) degrades gracefully.
import sys as _sys, types as _types
try:
    import antenv.axon_hooks  # noqa: F401
except Exception:
    _m = _types.ModuleType("antenv.axon_hooks")
    _m.get_axon_ntff_profile_hook = lambda: None
    _sys.modules["antenv.axon_hooks"] = _m

import ml_dtypes
import concourse.bacc as bacc
import concourse.bass as bass
import concourse.mybir as mybir
import concourse.tile as tile
from concourse.bass_utils import run_bass_kernel_spmd

F32 = mybir.dt.float32
BF16 = mybir.dt.bfloat16
I16 = mybir.dt.int16
AF = mybir.ActivationFunctionType

N_NODES = 50000
N_EDGES = 800000
FEAT = 90
H = 128
STEPS = 4
N_GRAPHS = 100
N_CORES = 8

N_AG = int(os.environ.get("K_NAG", "2"))    # AllGather pieces per step
N_SEG = max(N_AG, 2)                        # aggregation source segments
NC_NODES = N_NODES // N_CORES               # 6250 real nodes per core
N_PAD = 6656 if N_AG == 4 else 6400         # multiple of 128*N_AG and 512-ish
NT = N_PAD // 128                           # dst groups per core (50 or 52)
N_CHUNKS = (NT + 3) // 4                    # 13 chunks of <=4 groups
G_PADG = 112                                # padded global graph count
PAD_SLOT = 255.0                            # sentinel slot -> all-zero S row

if N_AG == 1:
    SEG_ROWS = N_CORES * N_PAD // 2         # address halves of one table
else:
    SEG_NODES = N_PAD // N_AG
    SEG_ROWS = N_CORES * SEG_NODES
assert SEG_ROWS <= 32768

TAB_DT_F32 = bool(int(os.environ.get("K_TAB_F32", "1")))
LAST_RESULTS = {}   # stash for test.py (exec time etc)


def _chunk_groups(ch):
    return range(ch * 4, min((ch + 1) * 4, NT))


# ----------------------------------------------------------------------------
# host-side layout
# ----------------------------------------------------------------------------

def _preprocess(x, edge_index, batch):
    batch = np.asarray(batch, np.int64)
    src = np.asarray(edge_index[0], np.int64)
    dst = np.asarray(edge_index[1], np.int64)
    frac = float(os.environ.get("K_EDGE_FRAC", "1"))
    if frac < 1.0:  # timing experiments only - wrong results
        n = int(len(src) * frac)
        src, dst = src[:n], dst[:n]

    d_core = dst // NC_NODES
    d_local = dst - d_core * NC_NODES
    grp = d_local // 128
    slot = (d_local % 128).astype(np.float32)
    s_core = src // NC_NODES
    s_local = src - s_core * NC_NODES

    if N_AG == 1:
        tpos = s_core * N_PAD + s_local
        seg = tpos // SEG_ROWS
        idxval = tpos - seg * SEG_ROWS
    else:
        seg = s_local // SEG_NODES
        idxval = s_core * SEG_NODES + (s_local - seg * SEG_NODES)

    # per (core, grp, seg) counts -> uniform budgets (min 1 tile so every
    # group is written by the pass-0 copy flush)
    cnt = np.zeros((N_CORES, NT, N_SEG), np.int64)
    np.add.at(cnt, (d_core, grp, seg), 1)
    budget = np.maximum(np.ceil(cnt.max(axis=0) / 128).astype(np.int64), 1)

    # tile order: seg-major, then chunk, then group
    tb = np.zeros((NT, N_SEG), np.int64)        # first tile of (grp, seg)
    chunk_t0 = np.zeros((N_CHUNKS, N_SEG), np.int64)
    chunk_nt = np.zeros((N_CHUNKS, N_SEG), np.int64)
    t = 0
    for s in range(N_SEG):
        for ch in range(N_CHUNKS):
            chunk_t0[ch, s] = t
            for g in _chunk_groups(ch):
                tb[g, s] = t
                t += int(budget[g, s])
            chunk_nt[ch, s] = t - chunk_t0[ch, s]
    t_tot = t

    # edge placement (vectorized)
    order = np.lexsort((grp, seg, d_core))
    sc = d_core[order]
    ss = seg[order]
    sg = grp[order]
    sidx = idxval[order]
    sslot = slot[order]
    rid = (sc * N_SEG + ss) * NT + sg
    run_first = np.r_[0, np.flatnonzero(np.diff(rid)) + 1]
    run_len = np.diff(np.r_[run_first, len(rid)])
    k = np.arange(len(rid)) - np.repeat(run_first, run_len)
    tt = tb[sg, ss] + k // 128
    pp = k % 128
    ct0 = chunk_t0[sg // 4, ss]
    pos = (tt - ct0) * 128 + pp

    idx_arr = np.zeros((N_CORES, 16, t_tot * 8), np.int16)
    slot_arr = np.full((N_CORES, 128, t_tot), PAD_SLOT, np.float32)
    idx_arr[sc, pos % 16, ct0 * 8 + pos // 16] = sidx.astype(np.int16)
    slot_arr[sc, pp, tt] = sslot

    # per-core node features (transposed, padded, bf16) and graph one-hots
    counts = np.bincount(batch, minlength=N_GRAPHS).astype(np.float32)
    invc = np.zeros((G_PADG, 1), np.float32)
    invc[:N_GRAPHS, 0] = 1.0 / np.maximum(counts, 1.0)
    x = np.asarray(x, np.float32)
    per_core = []
    for c in range(N_CORES):
        n0 = c * NC_NODES
        xT = np.zeros((FEAT, N_PAD), np.float32)
        xT[:, :NC_NODES] = x[n0:n0 + NC_NODES].T
        gmat = np.zeros((128, NT * G_PADG), np.float32)
        l = np.arange(NC_NODES)
        gmat[l % 128, (l // 128) * G_PADG + batch[n0:n0 + NC_NODES]] = 1.0
        per_core.append(dict(
            xT=xT.astype(ml_dtypes.bfloat16),
            idx=np.tile(idx_arr[c], (8, 1)),
            slot=slot_arr[c],
            gmat=gmat.astype(ml_dtypes.bfloat16),
        ))

    meta = dict(t_tot=t_tot, budget=budget, tb=tb,
                chunk_t0=chunk_t0, chunk_nt=chunk_nt, invc=invc)
    return per_core, meta


# ----------------------------------------------------------------------------
# device program
# ----------------------------------------------------------------------------

def _build(meta):
    DBG_STEPS = int(os.environ.get("K_STEPS", STEPS))
    DBG_NO_AG = bool(int(os.environ.get("K_NO_AG", "0")))
    DBG_NO_AGG = bool(int(os.environ.get("K_NO_AGG", "0")))
    DBG_NO_GRU = bool(int(os.environ.get("K_NO_GRU", "0")))
    t_tot = meta["t_tot"]
    budget = meta["budget"]
    tb = meta["tb"]
    chunk_t0 = meta["chunk_t0"]
    chunk_nt = meta["chunk_nt"]

    nc = bacc.Bacc("TRN2", target_bir_lowering=False, debug=False,
                   num_devices=N_CORES)

    d_xT = nc.dram_tensor("xT", [FEAT, N_PAD], BF16, kind="ExternalInput")
    d_idx = nc.dram_tensor("idx", [128, t_tot * 8], I16, kind="ExternalInput")
    d_slot = nc.dram_tensor("slot", [128, t_tot], F32, kind="ExternalInput")
    d_gmat = nc.dram_tensor("gmat", [128, NT * G_PADG], BF16,
                            kind="ExternalInput")
    d_invc = nc.dram_tensor("invc", [G_PADG, 1], F32, kind="ExternalInput")
    d_iota = nc.dram_tensor("iota", [1, 128], F32, kind="ExternalInput")
    d_ident = nc.dram_tensor("ident", [128, 128], BF16, kind="ExternalInput")
    d_wemb = nc.dram_tensor("wemb", [FEAT, H], BF16, kind="ExternalInput")
    d_wmsg = nc.dram_tensor("wmsg", [STEPS, H, H], BF16, kind="ExternalInput")
    d_wih = nc.dram_tensor("wih", [H, 3 * H], BF16, kind="ExternalInput")
    d_whh = nc.dram_tensor("whh", [H, 3 * H], BF16, kind="ExternalInput")
    d_bihT = nc.dram_tensor("bihT", [H, 3], F32, kind="ExternalInput")
    d_bhhT = nc.dram_tensor("bhhT", [H, 3], F32, kind="ExternalInput")
    d_w1 = nc.dram_tensor("w1", [H, H], BF16, kind="ExternalInput")
    d_b1 = nc.dram_tensor("b1", [H, 1], F32, kind="ExternalInput")
    d_w2 = nc.dram_tensor("w2", [H, 1], BF16, kind="ExternalInput")
    d_b2 = nc.dram_tensor("b2", [1, 1], F32, kind="ExternalInput")
    d_out = nc.dram_tensor("out", [1, G_PADG], F32, kind="ExternalOutput")

    with tile.TileContext(nc) as tc:
        with (
            tc.tile_pool(name="persist", bufs=1) as P,
            tc.tile_pool(name="dram", bufs=1, space="DRAM") as DR,
            tc.tile_pool(name="epool", bufs=3) as EP,
            tc.tile_pool(name="spool", bufs=8) as SP,
            tc.tile_pool(name="gpool", bufs=2) as GP,
            tc.tile_pool(name="ps_agg", bufs=2, space="PSUM") as PS_AGG,
            tc.tile_pool(name="ps_m", bufs=2, space="PSUM") as PS_M,
            tc.tile_pool(name="ps_gru", bufs=4, space="PSUM") as PS_GRU,
        ):
            # DRAM temps: message shards (by step parity) + per-step tables
            TDT = F32 if TAB_DT_F32 else BF16
            n_tab = max(DBG_STEPS, 1)
            if N_AG == 1:
                shard_bufs = [[DR.tile([N_PAD, H], TDT, name=f"m_shard{b}")]
                              for b in range(2)]
                table_bufs = [[DR.tile([2 * SEG_ROWS, H], TDT,
                                       addr_space="Shared", name=f"m_tab{st}")]
                              for st in range(n_tab)]
            else:
                shard_bufs = [[DR.tile([SEG_NODES, H], TDT,
                                       name=f"m_shard{b}_{q}")
                               for q in range(N_AG)] for b in range(2)]
                table_bufs = [[DR.tile([SEG_ROWS, H], TDT,
                                       addr_space="Shared",
                                       name=f"m_table{st}_{q}")
                               for q in range(N_AG)] for st in range(n_tab)]

            # ---------------- constants / weights ----------------
            slot_all = P.tile([128, t_tot], F32, name="slot_all")
            nc.sync.dma_start(out=slot_all[:], in_=d_slot[:, :])
            iota_f = P.tile([128, 128], F32, name="iota_f")
            nc.sync.dma_start(out=iota_f[:],
                              in_=d_iota.ap().to_broadcast([128, 128]))
            iota_b = P.tile([128, 128], BF16, name="iota_b")
            nc.vector.tensor_copy(iota_b[:], iota_f[:])
            ident_b = P.tile([128, 128], BF16, name="ident_b")
            nc.sync.dma_start(out=ident_b[:], in_=d_ident[:, :])

            def load(dram_ap, shape, name, dt=BF16):
                tl = P.tile(shape, dt, name=name)
                nc.sync.dma_start(out=tl[:], in_=dram_ap)
                return tl

            wemb_b = load(d_wemb[:, :], [FEAT, H], "wemb")
            wmsg_b = [load(d_wmsg[s, :, :], [H, H], f"wmsg{s}")
                      for s in range(STEPS)]
            wih_b = load(d_wih[:, :], [H, 3 * H], "wih")
            whh_b = load(d_whh[:, :], [H, 3 * H], "whh")
            w1_b = load(d_w1[:, :], [H, H], "w1")
            w2_b = load(d_w2[:, :], [H, 1], "w2")
            bih = load(d_bihT[:, :], [H, 3], "bih", F32)
            bhh = load(d_bhhT[:, :], [H, 3], "bhh", F32)
            bsum = P.tile([H, 3], F32, name="bsum")
            nc.vector.tensor_add(bsum[:], bih[:], bhh[:])
            b1t = load(d_b1[:, :], [H, 1], "b1t", F32)
            b2t = load(d_b2[:, :], [1, 1], "b2t", F32)
            invc_t = load(d_invc[:, :], [G_PADG, 1], "invc_t", F32)
            gmat_b = load(d_gmat[:, :], [128, NT * G_PADG], "gmat")

            # state
            h_t = P.tile([128, N_PAD], BF16, name="h_t")
            m_all = P.tile([128, N_PAD], F32 if TAB_DT_F32 else BF16,
                           name="m_all")
            aggT = P.tile([128, N_PAD], BF16, name="aggT")

            def msg_tile(t, step):
                pm = PS_M.tile([128, 128], F32, name="pm", tag="pm")
                nc.tensor.matmul(pm[:], lhsT=h_t[:, t * 128:(t + 1) * 128],
                                 rhs=wmsg_b[step % STEPS][:, :],
                                 start=True, stop=True)
                nc.scalar.activation(m_all[:, t * 128:(t + 1) * 128],
                                     pm[:], AF.Copy)

            def send_seg(step, q):
                """DMA m_all segment q to its shard and AllGather it."""
                shard = shard_bufs[step % 2][0 if N_AG == 1 else q]
                if N_AG == 1:
                    src = m_all[:]
                else:
                    src = m_all[:, q * SEG_NODES:(q + 1) * SEG_NODES]
                nc.sync.dma_start(
                    out=shard.rearrange("(a p) b -> p a b", p=128),
                    in_=src.rearrange("p (a b) -> p a b", b=128))
                if not DBG_NO_AG:
                    nc.gpsimd.collective_compute(
                        "AllGather", mybir.AluOpType.bypass,
                        ins=[shard.opt()],
                        outs=[table_bufs[step][0 if N_AG == 1 else q].opt()],
                        replica_groups=[list(range(N_CORES))],
                    )

            # chunk index after which message tiles for AG piece q are done
            msg_seg_tiles = NT // N_AG
            ag_after_chunk = {}
            for q in range(N_AG):
                last_tile = (q + 1) * msg_seg_tiles - 1
                ag_after_chunk.setdefault(last_tile // 4, []).append(q)

            def gru_chunk(ch, step):
                off = ch * 512
                size = min(512, N_PAD - off)
                sl = slice(off, off + size)
                p_r = PS_GRU.tile([128, size], F32, name="p_r", tag="pgru")
                nc.tensor.matmul(p_r[:], lhsT=wih_b[:, 0:128],
                                 rhs=aggT[:, sl], start=True, stop=False)
                nc.tensor.matmul(p_r[:], lhsT=whh_b[:, 0:128],
                                 rhs=h_t[:, sl], start=False, stop=True)
                p_z = PS_GRU.tile([128, size], F32, name="p_z", tag="pgru")
                nc.tensor.matmul(p_z[:], lhsT=wih_b[:, 128:256],
                                 rhs=aggT[:, sl], start=True, stop=False)
                nc.tensor.matmul(p_z[:], lhsT=whh_b[:, 128:256],
                                 rhs=h_t[:, sl], start=False, stop=True)
                p_xn = PS_GRU.tile([128, size], F32, name="p_xn", tag="pgru")
                nc.tensor.matmul(p_xn[:], lhsT=wih_b[:, 256:384],
                                 rhs=aggT[:, sl], start=True, stop=True)
                p_hn = PS_GRU.tile([128, size], F32, name="p_hn", tag="pgru")
                nc.tensor.matmul(p_hn[:], lhsT=whh_b[:, 256:384],
                                 rhs=h_t[:, sl], start=True, stop=True)
                r_t = GP.tile([128, size], BF16, name="r_t", tag="gp1")
                nc.scalar.activation(r_t[:], p_r[:], AF.Sigmoid,
                                     bias=bsum[:, 0:1])
                z_t = GP.tile([128, size], BF16, name="z_t", tag="gp2")
                nc.scalar.activation(z_t[:], p_z[:], AF.Sigmoid,
                                     bias=bsum[:, 1:2])
                hn_t = GP.tile([128, size], BF16, name="hn_t", tag="gp3")
                nc.scalar.activation(hn_t[:], p_hn[:], AF.Identity,
                                     bias=bhh[:, 2:3])
                t1 = GP.tile([128, size], BF16, name="t1", tag="gp4")
                nc.vector.tensor_mul(t1[:], r_t[:], hn_t[:])
                u_t = GP.tile([128, size], F32, name="u_t", tag="gp5")
                nc.vector.tensor_add(u_t[:], t1[:], p_xn[:])
                n_t = GP.tile([128, size], F32, name="n_t", tag="gp6")
                nc.scalar.activation(n_t[:], u_t[:], AF.Tanh,
                                     bias=bih[:, 2:3])
                d_t = GP.tile([128, size], F32, name="d_t", tag="gp7")
                nc.vector.tensor_sub(d_t[:], h_t[:, sl], n_t[:])
                e_t = GP.tile([128, size], F32, name="e_t", tag="gp8")
                nc.vector.tensor_mul(e_t[:], z_t[:], d_t[:])
                nc.vector.tensor_add(h_t[:, sl], n_t[:], e_t[:])

            # ---------------- embedding ----------------
            xT_b = P.tile([FEAT, N_PAD], BF16, name="xT_b")
            nc.sync.dma_start(out=xT_b[:], in_=d_xT[:, :])
            for ch in range(N_CHUNKS):
                off = ch * 512
                size = min(512, N_PAD - off)
                pe = PS_GRU.tile([128, size], F32, name="pe_emb", tag="pgru")
                nc.tensor.matmul(pe[:], lhsT=wemb_b[:, :],
                                 rhs=xT_b[:, off:off + size],
                                 start=True, stop=True)
                nc.scalar.activation(h_t[:, off:off + size], pe[:], AF.Relu)

            # ---------------- message-passing steps ----------------
            if DBG_STEPS > 0:
                for t in range(NT):
                    msg_tile(t, 0)
                for q in range(N_AG):
                    send_seg(0, q)

            for step in range(DBG_STEPS):
                tabs = table_bufs[step]
                if N_AG == 1:
                    tab_half = [tabs[0][0:SEG_ROWS, :],
                                tabs[0][SEG_ROWS:2 * SEG_ROWS, :]]
                else:
                    tab_half = [tabs[q][:, :] for q in range(N_AG)]

                if DBG_NO_AGG:
                    nc.vector.memset(aggT[:], 0.0)
                for s in range(N_SEG):
                    last = s == N_SEG - 1
                    for ch in range(N_CHUNKS):
                        if not DBG_NO_AGG:
                            t0 = int(chunk_t0[ch, s])
                            n_ch = int(chunk_nt[ch, s])
                            ix = EP.tile([128, n_ch * 8], I16, name="ix",
                                         tag="ix")
                            nc.sync.dma_start(
                                out=ix[:],
                                in_=d_idx[:, t0 * 8:(t0 + n_ch) * 8])
                            E = EP.tile([128, n_ch, 128],
                                        F32 if TAB_DT_F32 else BF16,
                                        name="E", tag="E")
                            nc.gpsimd.dma_gather(
                                E[:], tab_half[s], ix[:],
                                n_ch * 128, n_ch * 128, H,
                                single_packet=bool(int(
                                    os.environ.get("K_SP", "0"))))
                            for g in _chunk_groups(ch):
                                nb = int(budget[g, s])
                                pa = PS_AGG.tile([128, 128], F32, name="pa",
                                                 tag="pa")
                                for kk in range(nb):
                                    tg = int(tb[g, s]) + kk
                                    St = SP.tile([128, 128],
                                                 F32 if TAB_DT_F32 else BF16,
                                                 name="St", tag="St")
                                    nc.vector.tensor_scalar(
                                        St[:], iota_f[:]
                                        if TAB_DT_F32 else iota_b[:],
                                        slot_all[:, tg:tg + 1], None,
                                        mybir.AluOpType.is_equal)
                                    nc.tensor.matmul(
                                        pa[:], lhsT=E[:, tg - t0, :],
                                        rhs=St[:],
                                        start=(kk == 0), stop=(kk == nb - 1))
                                sl = slice(g * 128, (g + 1) * 128)
                                if s == 0:
                                    nc.scalar.activation(aggT[:, sl], pa[:],
                                                         AF.Copy)
                                else:
                                    nc.vector.tensor_add(aggT[:, sl],
                                                         aggT[:, sl], pa[:])
                        if last:
                            if not DBG_NO_GRU:
                                gru_chunk(ch, step)
                            if step + 1 < DBG_STEPS:
                                for t in range(ch * 4,
                                               min((ch + 1) * 4, NT)):
                                    msg_tile(t, step + 1)
                                for q in ag_after_chunk.get(ch, []):
                                    send_seg(step + 1, q)

            # ---------------- readout ----------------
            hr = P.tile([128, N_PAD], BF16, name="hr")
            for ch in range(N_CHUNKS):
                off = ch * 512
                size = min(512, N_PAD - off)
                nc.scalar.activation(hr[:, off:off + size],
                                     h_t[:, off:off + size], AF.Relu)
            pp = PS_GRU.tile([G_PADG, 128], F32, name="pp", tag="pgru")
            for t in range(NT):
                ptr2 = PS_M.tile([128, 128], BF16, name="ptr2", tag="pm")
                nc.tensor.transpose(ptr2[:], hr[:, t * 128:(t + 1) * 128],
                                    ident_b[:])
                hnm = GP.tile([128, 128], BF16, name="hnm", tag="gp1")
                nc.scalar.activation(hnm[:], ptr2[:], AF.Copy)
                nc.tensor.matmul(pp[:],
                                 lhsT=gmat_b[:, t * G_PADG:(t + 1) * G_PADG],
                                 rhs=hnm[:], start=(t == 0), stop=(t == NT - 1))
            poolp = P.tile([G_PADG, 128], F32, name="poolp")
            nc.scalar.activation(poolp[:], pp[:], AF.Copy)
            d_pool_in = DR.tile([G_PADG, H], F32, name="pool_in")
            d_pool_out = DR.tile([G_PADG, H], F32, addr_space="Shared",
                                 name="pool_out")
            nc.sync.dma_start(out=d_pool_in[:, :], in_=poolp[:])
            nc.gpsimd.collective_compute(
                "AllReduce", mybir.AluOpType.add,
                ins=[d_pool_in.opt()], outs=[d_pool_out.opt()],
                replica_groups=[list(range(N_CORES))],
            )
            pool_r = P.tile([G_PADG, 128], F32, name="pool_r")
            nc.sync.dma_start(out=pool_r[:], in_=d_pool_out[:, :])
            pooled = P.tile([G_PADG, 128], BF16, name="pooled")
            nc.vector.tensor_scalar(pooled[:], pool_r[:], invc_t[:], None,
                                    mybir.AluOpType.mult)
            ppt = PS_M.tile([128, G_PADG], BF16, name="ppt", tag="pm")
            nc.tensor.transpose(ppt[:], pooled[:],
                                ident_b[0:G_PADG, 0:G_PADG])
            pooledT = P.tile([128, G_PADG], BF16, name="pooledT")
            nc.scalar.activation(pooledT[:], ppt[:], AF.Copy)
            pz1 = PS_M.tile([128, G_PADG], F32, name="pz1", tag="pm")
            nc.tensor.matmul(pz1[:], lhsT=w1_b[:, :], rhs=pooledT[:],
                             start=True, stop=True)
            z1 = P.tile([128, G_PADG], BF16, name="z1")
            nc.scalar.activation(z1[:], pz1[:], AF.Relu, bias=b1t[:, 0:1])
            po = PS_M.tile([1, G_PADG], F32, name="po", tag="pm")
            nc.tensor.matmul(po[:], lhsT=w2_b[:, :], rhs=z1[:],
                             start=True, stop=True)
            esb = P.tile([1, G_PADG], F32, name="esb")
            nc.scalar.activation(esb[:], po[:], AF.Exp, bias=b2t[:, 0:1])
            osb = P.tile([1, G_PADG], F32, name="osb")
            nc.scalar.activation(osb[:], esb[:], AF.Ln, bias=1.0)
            nc.sync.dma_start(out=d_out[:, :], in_=osb[:])

    nc.compile()
    return nc


# ----------------------------------------------------------------------------
# entry point
# ----------------------------------------------------------------------------

def make_in_maps(inputs, per_core, meta):
    return _make_in_maps(per_core, meta, **{
        k: inputs[k] for k in ("W_emb", "W_msg", "W_ih", "W_hh", "b_ih",
                               "b_hh", "W1", "b1", "W2", "b2")})


def _make_in_maps(per_core, meta, W_emb, W_msg, W_ih, W_hh, b_ih, b_hh,
                  W1, b1, W2, b2):
    bf = ml_dtypes.bfloat16
    shared = dict(
        iota=np.arange(128, dtype=np.float32).reshape(1, 128),
        ident=np.eye(128, dtype=np.float32).astype(bf),
        wemb=np.asarray(W_emb, np.float32).astype(bf),
        wmsg=np.asarray(W_msg, np.float32).astype(bf),
        wih=np.asarray(W_ih, np.float32).astype(bf),
        whh=np.asarray(W_hh, np.float32).astype(bf),
        bihT=np.ascontiguousarray(
            np.asarray(b_ih, np.float32).reshape(3, H).T),
        bhhT=np.ascontiguousarray(
            np.asarray(b_hh, np.float32).reshape(3, H).T),
        w1=np.asarray(W1, np.float32).astype(bf),
        b1=np.asarray(b1, np.float32).reshape(H, 1),
        w2=np.asarray(W2, np.float32).astype(bf),
        b2=np.asarray(b2, np.float32).reshape(1, 1),
        invc=meta["invc"],
    )
    in_maps = []
    for c in range(N_CORES):
        m = dict(shared)
        m["xT"] = per_core[c]["xT"]
        m["idx"] = per_core[c]["idx"]
        m["slot"] = per_core[c]["slot"]
        m["gmat"] = per_core[c]["gmat"]
        in_maps.append(m)
    return in_maps


def kernel(x, edge_index, batch, W_emb, W_msg, W_ih, W_hh, b_ih, b_hh,
           W1, b1, W2, b2):
    per_core, meta = _preprocess(x, edge_index, batch)
    nc = _build(meta)
    in_maps = _make_in_maps(per_core, meta, W_emb, W_msg, W_ih, W_hh,
                            b_ih, b_hh, W1, b1, W2, b2)

    trace = bool(int(os.environ.get("KERNEL_TRACE", "0")))
    res = run_bass_kernel_spmd(nc, in_maps, list(range(N_CORES)), trace=trace)
    LAST_RESULTS["exec_time_ns"] = res.exec_time_ns
    LAST_RESULTS["profile_json"] = res.profile_json
    LAST_RESULTS["nc"] = nc
    LAST_RESULTS["in_maps"] = in_maps

    return np.asarray(res.results[0]["out"][0, :N_GRAPHS], np.float32)
